# revision 1
# baseline (speedup 1.0000x reference)
"""DebertaV2 disentangled attention block on 8 TRN2 NeuronCores (Bass/Tile).

Head-sharded tensor parallel: 2 heads per core. Host does layout-only prep
(transpose / bucket-reversal / dtype cast); all FLOPs run on device.
ReduceScatter after the output dense; per-core LayerNorm on its 128 rows.
"""

import math

import numpy as np

H = 16
D = 64
HID = 1024
N = 1024
K = 1024
EPS = 1e-7
NCORES = 8
HPC = H // NCORES  # heads per core = 2
DPC = HPC * D      # head dims per core = 128
SCALE = 1.0 / math.sqrt(3.0 * D)  # applied inside exp()

W_WIN = 1151       # skew window width (127 + 1024)
P = 128

_CACHE = {}


def _build():
    import concourse.bass as bass
    import concourse.mybir as mybir
    import concourse.tile as tile
    from concourse import bacc
    from concourse.masks import make_identity
    from contextlib import ExitStack

    f32 = mybir.dt.float32
    bf16 = mybir.dt.bfloat16

    nc = bacc.Bacc(None, target_bir_lowering=False, debug=False)
    names = {}

    with tile.TileContext(nc) as tc, ExitStack() as es:
        dio = es.enter_context(tc.tile_pool(name="dram_io", bufs=1, space="DRAM"))
        dwork = es.enter_context(tc.tile_pool(name="dram_work", bufs=1, space="DRAM"))

        def din(nm, shape, dt=bf16):
            t = dio.tile(shape, dt, kind="ExternalInput", name=nm, tag=nm)
            names[nm] = t.name
            return t

        hsT = din("hsT", (HID, N))            # hs[0].T, bf16
        relTr = din("relTr", (HID, 2 * K))    # rel[::-1].T, bf16 (for pos_k)
        relTn = din("relTn", (HID, 2 * K))    # rel.T, bf16 (for pos_q)
        wqT = din("wqT", (HID, DPC))
        wkT = din("wkT", (HID, DPC))
        wvT = din("wvT", (HID, DPC))
        wpkT = din("wpkT", (HID, DPC))
        wpqT = din("wpqT", (HID, DPC))
        woT = din("woT", (DPC, HID))
        hs_rows = din("hs_rows", (P, HID), f32)
        bq_s = din("bq_s", (DPC,), f32)
        bk_s = din("bk_s", (DPC,), f32)
        bv_s = din("bv_s", (DPC,), f32)
        bpk_s = din("bpk_s", (DPC,), f32)
        bpq_s = din("bpq_s", (DPC,), f32)
        bo_t = din("bo", (HID,), f32)
        lng_t = din("ln_g", (HID,), f32)
        lnb_t = din("ln_b", (HID,), f32)

        out_t = dio.tile((P, HID), f32, kind="ExternalOutput", name="out", tag="out")
        names["out"] = out_t.name

        opart = dwork.tile((N, HID), bf16, name="opart", tag="opart")
        rs_out = dwork.tile((P, HID), bf16, name="rs_out", tag="rs_out")

        # ---- SBUF pools -------------------------------------------------
        wt = es.enter_context(tc.tile_pool(name="wt", bufs=1))
        work = es.enter_context(tc.tile_pool(name="work", bufs=1))
        psA = es.enter_context(tc.tile_pool(name="psA", bufs=6, space="PSUM"))
        psB = es.enter_context(tc.tile_pool(name="psB", bufs=1, space="PSUM"))

        Iden = mybir.ActivationFunctionType.Identity
        Exp = mybir.ActivationFunctionType.Exp
        Sqrt = mybir.ActivationFunctionType.Sqrt
        ADD = mybir.AluOpType.add
        MUL = mybir.AluOpType.mult
        SUB = mybir.AluOpType.subtract

        # ---- persistent small inputs ------------------------------------
        w_sb = {k: [] for k in ("q", "k", "v", "pk", "pq")}
        wmap = {"q": wqT, "k": wkT, "v": wvT, "pk": wpkT, "pq": wpqT}
        for t in range(8):
            for kk in w_sb:
                c = wt.tile([P, DPC], bf16, name=f"w{kk}{t}", tag=f"w{kk}{t}")
                nc.sync.dma_start(c[:], wmap[kk][128 * t:128 * (t + 1), :])
                w_sb[kk].append(c)
        woT_sb = wt.tile([P, HID], bf16, name="woT_sb", tag="woT_sb")
        nc.sync.dma_start(woT_sb[:], woT[:])

        def bias_tile(nm, src, n=DPC):
            t = wt.tile([n, 1], f32, name=nm, tag=nm)
            nc.sync.dma_start(t[:], bass.AP(src[:].tensor, src[:].offset, [[1, n]]))
            return t

        bq_sb = bias_tile("bq_sb", bq_s)
        bk_sb = bias_tile("bk_sb", bk_s)
        bv_sb = bias_tile("bv_sb", bv_s)
        bpk_sb = bias_tile("bpk_sb", bpk_s)
        bpq_sb = bias_tile("bpq_sb", bpq_s)

        def bcast_tile(nm, src, dt):
            t = wt.tile([P, HID], dt, name=nm, tag=nm)
            if dt == f32:
                nc.sync.dma_start(t[:], bass.AP(src[:].tensor, src[:].offset,
                                                [[0, P], [1, HID]]))
            else:
                nc.gpsimd.dma_start(t[:], bass.AP(src[:].tensor, src[:].offset,
                                                  [[0, P], [1, HID]]))
            return t

        bo_bc = bcast_tile("bo_bc", bo_t, f32)
        g_bc = bcast_tile("g_bc", lng_t, f32)
        b_bc = bcast_tile("b_bc", lnb_t, f32)

        hsr_sb = wt.tile([P, HID], f32, name="hsr_sb", tag="hsr_sb")
        nc.sync.dma_start(hsr_sb[:], hs_rows[:])

        ident = wt.tile([P, P], bf16, name="ident", tag="ident")
        make_identity(nc, ident[:])

        # ---- projections -------------------------------------------------
        qT = wt.tile([P, N], bf16, name="qT", tag="qT")
        kT = wt.tile([P, N], bf16, name="kT", tag="kT")
        pkT = wt.tile([P, 2 * K], bf16, name="pkT", tag="pkT")
        pqT = wt.tile([P, 2 * K], bf16, name="pqT", tag="pqT")

        def load_tiles(src, nt, width, nm):
            tiles, frees = [], []
            for t in range(nt):
                a, fa = tc.tile([P, width], bf16, name=f"{nm}{t}")
                nc.sync.dma_start(a[:], src[128 * t:128 * (t + 1), :])
                tiles.append(a)
                frees.append(fa)
            return tiles, frees

        def project(dst, wlist, rhs_list, width, bias):
            for c0 in range(0, width, 512):
                ps = psA.tile([P, 512], f32, name="pp", tag="pp")
                for t in range(8):
                    nc.tensor.matmul(ps[:], wlist[t][:],
                                     rhs_list[t][:, c0:c0 + 512],
                                     start=(t == 0), stop=(t == 7))
                nc.scalar.activation(dst[:, c0:c0 + 512], ps[:], Iden,
                                     bias=bias[:])

        hsT_sb, hsT_free = load_tiles(hsT, 8, N, "hsT")
        project(qT, w_sb["q"], hsT_sb, N, bq_sb)
        project(kT, w_sb["k"], hsT_sb, N, bk_sb)

        # v in [j, d] layout + ones column per head: va[jt] is [128, 132]
        va = []
        for jt in range(8):
            t = wt.tile([P, 132], bf16, name=f"va{jt}", tag=f"va{jt}")
            ps = psA.tile([P, DPC], f32, name="pv", tag="pp")
            for kt in range(8):
                nc.tensor.matmul(ps[:], hsT_sb[kt][:, 128 * jt:128 * (jt + 1)],
                                 w_sb["v"][kt][:], start=(kt == 0), stop=(kt == 7))
            nc.scalar.copy(t[:, 0:64], ps[:, 0:64])
            nc.scalar.copy(t[:, 66:130], ps[:, 64:128])
            nc.vector.memset(t[:, 64:65], 1.0)
            nc.vector.memset(t[:, 130:131], 1.0)
            va.append(t)
        for f in reversed(hsT_free):
            f()

        relTr_sb, relTr_free = load_tiles(relTr, 8, 2 * K, "relTr")
        project(pkT, w_sb["pk"], relTr_sb, 2 * K, bpk_sb)
        for f in reversed(relTr_free):
            f()
        relTn_sb, relTn_free = load_tiles(relTn, 8, 2 * K, "relTn")
        project(pqT, w_sb["pq"], relTn_sb, 2 * K, bpq_sb)
        for f in reversed(relTn_free):
            f()

        # ---- attention per head -----------------------------------------
        ctxT = wt.tile([P, N], bf16, name="ctxT", tag="ctxT")

        def skew_block(lhs, src_T, hd, idx, tagp, tag, bufs):
            """blk[p, c] = lhs[hd][:, 128*idx+p] . src_T[hd][:, w0+c]
            -> dst[p, x] = blk[p, 127 - p + x]   (shape [128, 1024])"""
            w0 = (896 if tagp == "c" else 897) - 128 * idx
            blk = work.tile([P, W_WIN], bf16, name=f"blk_{tagp}{idx}",
                            tag="blk", bufs=3)
            for (c0, w) in ((0, 512), (512, 512), (1024, 127)):
                ps = psA.tile([P, 512], f32, name="pblk", tag="pp")
                nc.tensor.matmul(
                    ps[:, 0:w],
                    lhs[hd, 128 * idx:128 * (idx + 1)],
                    src_T[hd, w0 + c0:w0 + c0 + w],
                    start=True, stop=True)
                if tagp == "c":
                    nc.vector.tensor_copy(blk[:, c0:c0 + w], ps[:, 0:w])
                else:
                    nc.scalar.copy(blk[:, c0:c0 + w], ps[:, 0:w])
            scr = dwork.tile((P * W_WIN,), bf16, name=f"scr_{tagp}{idx}",
                             tag="scr", bufs=4)
            h = scr[:].tensor
            nc.sync.dma_start(
                bass.AP(h, scr[:].offset, [[W_WIN, P], [1, W_WIN]]), blk[:])
            dst = work.tile([P, N], bf16, name=f"g_{tagp}{idx}", tag=tag,
                            bufs=bufs)
            nc.sync.dma_start(
                dst[:], bass.AP(h, scr[:].offset + 127, [[W_WIN - 1, P], [1, N]]))
            return dst

        for h in range(HPC):
            hd = slice(64 * h, 64 * h + 64)
            # c2p gathered tiles, one per i-tile r: [128 i, 1024 j]
            c2p = [skew_block(qT, pkT, hd, r, "c", f"g_c{r}", 1)
                   for r in range(8)]

            pb = psB.tile([65, N], f32, name="pb", tag="pb")
            for jt in range(8):
                # p2cT tile for this j-tile: [128 j, 1024 i]
                p2cT = skew_block(kT, pqT, hd, jt, "p", "g_p", 2)
                e = work.tile([P, N], bf16, name=f"expST{jt}", tag="expST",
                              bufs=2)
                for c in range(2):
                    st = psA.tile([P, 512], f32, name="st", tag="pp")
                    nc.tensor.matmul(st[:], kT[hd, 128 * jt:128 * (jt + 1)],
                                     qT[hd, 512 * c:512 * (c + 1)],
                                     start=True, stop=False)
                    for rr in range(4):
                        r = 4 * c + rr
                        nc.tensor.matmul(st[:, 128 * rr:128 * (rr + 1)],
                                         c2p[r][:, 128 * jt:128 * (jt + 1)],
                                         ident[:], start=False, stop=(rr == 3))
                    s_sb = work.tile([P, 512], f32, name="s_sb", tag="s_sb",
                                     bufs=3)
                    nc.vector.tensor_add(s_sb[:], st[:],
                                         p2cT[:, 512 * c:512 * (c + 1)])
                    nc.scalar.activation(e[:, 512 * c:512 * (c + 1)], s_sb[:],
                                         Exp, scale=SCALE)
                for c in range(2):
                    nc.tensor.matmul(pb[:, 512 * c:512 * (c + 1)],
                                     va[jt][:, 66 * h:66 * h + 65],
                                     e[:, 512 * c:512 * (c + 1)],
                                     start=(jt == 0), stop=(jt == 7))

            recip = work.tile([1, N], f32, name="recip", tag="recip", bufs=2)
            nc.vector.reciprocal(recip[:], pb[64:65, :])
            rscr = dwork.tile((N,), f32, name=f"rscr{h}", tag="rscr", bufs=2)
            rh = rscr[:].tensor
            nc.sync.dma_start(bass.AP(rh, rscr[:].offset, [[1, N]]), recip[:])
            rbc = work.tile([64, N], f32, name="rbc", tag="rbc", bufs=2)
            nc.sync.dma_start(rbc[:], bass.AP(rh, rscr[:].offset, [[0, 64], [1, N]]))
            ctmp = work.tile([64, N], bf16, name="ctmp", tag="ctmp", bufs=2)
            nc.vector.tensor_mul(ctmp[:], pb[0:64, :], rbc[:])
            nc.scalar.activation(ctxT[hd, :], ctmp[:], Iden, bias=bv_sb[hd, :])

        # ---- output dense (partial) -> DRAM ------------------------------
        for it in range(8):
            osb = work.tile([P, HID], bf16, name="osb", tag="osb", bufs=2)
            for c in range(2):
                po = psA.tile([P, 512], f32, name="po", tag="pp")
                nc.tensor.matmul(po[:], ctxT[:, 128 * it:128 * (it + 1)],
                                 woT_sb[:, 512 * c:512 * (c + 1)],
                                 start=True, stop=True)
                nc.scalar.copy(osb[:, 512 * c:512 * (c + 1)], po[:])
            nc.sync.dma_start(opart[128 * it:128 * (it + 1), :], osb[:])

        # ---- ReduceScatter ------------------------------------------------
        nc.gpsimd.collective_compute(
            "ReduceScatter", ADD, replica_groups=[list(range(NCORES))],
            ins=[opart[:]], outs=[rs_out[:]])

        # ---- residual + LayerNorm on this core's 128 rows ----------------
        xr = wt.tile([P, HID], f32, name="xr", tag="xr")
        nc.gpsimd.dma_start(xr[:], rs_out[:])  # bf16 -> f32 cast dma
        x = wt.tile([P, HID], f32, name="x", tag="x")
        nc.vector.tensor_add(x[:], xr[:], hsr_sb[:])
        nc.vector.tensor_add(x[:], x[:], bo_bc[:])

        stats = wt.tile([P, 2, 6], f32, name="stats", tag="stats")
        mv = wt.tile([P, 2], f32, name="mv", tag="mv")
        for s in range(2):
            nc.vector.bn_stats(stats[:, s, :], x[:, 512 * s:512 * (s + 1)])
        nc.vector.bn_aggr(mv[:], stats[:])
        epsb = wt.tile([P, 1], f32, name="epsb", tag="epsb")
        nc.vector.memset(epsb[:], EPS)
        std = wt.tile([P, 1], f32, name="std", tag="std")
        nc.scalar.activation(std[:], mv[:, 1:2], Sqrt, bias=epsb[:])
        rstd = wt.tile([P, 1], f32, name="rstd", tag="rstd")
        nc.vector.reciprocal(rstd[:], std[:])

        t1 = wt.tile([P, HID], f32, name="t1", tag="t1")
        nc.vector.scalar_tensor_tensor(t1[:], x[:], mv[:, 0:1], g_bc[:],
                                       op0=SUB, op1=MUL)
        yout = wt.tile([P, HID], f32, name="yout", tag="yout")
        nc.vector.scalar_tensor_tensor(yout[:], t1[:], rstd[:], b_bc[:],
                                       op0=MUL, op1=ADD)
        nc.sync.dma_start(out_t[:], yout[:])

    nc.compile()
    return nc, names


def _get_compiled():
    if "nc" not in _CACHE:
        nc, names = _build()
        _CACHE["nc"] = nc
        _CACHE["names"] = names
    return _CACHE["nc"], _CACHE["names"]


def _prep_in_maps(inputs):
    import ml_dtypes

    bf = ml_dtypes.bfloat16
    hs = np.asarray(inputs["hidden_states"], np.float32)[0]      # (N, HID)
    rel = np.asarray(inputs["rel_embeddings"], np.float32)       # (2K, HID)
    hsT = np.ascontiguousarray(hs.T).astype(bf)
    relTr = np.ascontiguousarray(rel[::-1].T).astype(bf)
    relTn = np.ascontiguousarray(rel.T).astype(bf)

    def wT(w, r):
        w = np.asarray(w, np.float32)
        return np.ascontiguousarray(w[DPC * r:DPC * (r + 1), :].T).astype(bf)

    in_maps = []
    for r in range(NCORES):
        m = {
            "hsT": hsT,
            "relTr": relTr,
            "relTn": relTn,
            "wqT": wT(inputs["Wq"], r),
            "wkT": wT(inputs["Wk"], r),
            "wvT": wT(inputs["Wv"], r),
            "wpkT": wT(inputs["Wpk"], r),
            "wpqT": wT(inputs["Wpq"], r),
            "woT": np.ascontiguousarray(
                np.asarray(inputs["Wo"], np.float32)[:, DPC * r:DPC * (r + 1)].T
            ).astype(bf),
            "hs_rows": np.ascontiguousarray(hs[P * r:P * (r + 1), :]),
            "bq_s": np.asarray(inputs["bq"], np.float32)[DPC * r:DPC * (r + 1)],
            "bk_s": np.asarray(inputs["bk"], np.float32)[DPC * r:DPC * (r + 1)],
            "bv_s": np.asarray(inputs["bv"], np.float32)[DPC * r:DPC * (r + 1)],
            "bpk_s": np.asarray(inputs["bpk"], np.float32)[DPC * r:DPC * (r + 1)],
            "bpq_s": np.asarray(inputs["bpq"], np.float32)[DPC * r:DPC * (r + 1)],
            "bo": np.asarray(inputs["bo"], np.float32),
            "ln_g": np.asarray(inputs["ln_g"], np.float32),
            "ln_b": np.asarray(inputs["ln_b"], np.float32),
        }
        in_maps.append(m)
    return in_maps


def run(inputs, trace=False):
    from concourse.bass_utils import run_bass_kernel_spmd

    nc, names = _get_compiled()
    logical = _prep_in_maps(inputs)
    in_maps = [{names[k]: v for k, v in m.items()} for m in logical]
    res = run_bass_kernel_spmd(nc, in_maps, list(range(NCORES)), trace=trace)
    outs = [res.results[r][names["out"]].astype(np.float32) for r in range(NCORES)]
    full = np.concatenate(outs, axis=0).reshape(1, N, HID)
    return full, res


def kernel(**inputs) -> np.ndarray:
    full, _ = run(inputs, trace=False)
    return full



# revision 2
# speedup vs baseline: 1.1034x; 1.1034x over previous
"""DebertaV2 disentangled attention, 8 TRN2 cores (Bass/Tile), v2.

Head-sharded TP (2 heads/core). Single rel load (pk projection written
column-reversed during PSUM evacuation), host-packed weights, batched skew
bounces, AllToAll for the output resharding, per-core LayerNorm on 128 rows.
"""

import math

import numpy as np

H = 16
D = 64
HID = 1024
N = 1024
K = 1024
EPS = 1e-7
NCORES = 8
HPC = H // NCORES   # heads per core = 2
DPC = HPC * D       # head dims per core = 128
SCALE = 1.0 / math.sqrt(3.0 * D)

W_WIN = 1151        # skew window width (127 + 1024)
P = 128
FP8_SKEW = True

_CACHE = {}


def _build():
    import concourse.bass as bass
    import concourse.mybir as mybir
    import concourse.tile as tile
    from concourse import bacc
    from concourse.masks import make_identity
    from contextlib import ExitStack

    f32 = mybir.dt.float32
    bf16 = mybir.dt.bfloat16
    skew_dt = mybir.dt.float8e4 if FP8_SKEW else bf16

    nc = bacc.Bacc(None, target_bir_lowering=False, debug=False)
    names = {}

    with tile.TileContext(nc) as tc, ExitStack() as es:
        dio = es.enter_context(tc.tile_pool(name="dram_io", bufs=1, space="DRAM"))
        dwork = es.enter_context(tc.tile_pool(name="dram_work", bufs=1, space="DRAM"))

        def din(nm, shape, dt=bf16):
            t = dio.tile(shape, dt, kind="ExternalInput", name=nm, tag=nm)
            names[nm] = t.name
            return t

        hst = din("hst", (P, 8 * N))          # hs.T packed: [p, kt*1024+c]
        relT = din("relT", (P, 8 * 2 * K))    # rel.T packed: [p, kt*2048+c]
        wpack = din("wpack", (P, 5 * 8 * P))  # q,k,v,pk,pq kt-blocks
        wot = din("wot", (DPC, HID))          # Wo.T slice for this core
        hsr = din("hsr", (P, HID), f32)       # hs rows + bo (host-folded)
        bias8 = din("bias8", (P, 8), f32)     # cols: bq,bk,bv,bpk,bpq
        gl = din("gl", (1, 2 * HID), f32)     # [ln_g | ln_b]

        out_t = dio.tile((P, HID), f32, kind="ExternalOutput", name="out", tag="out")
        names["out"] = out_t.name

        # ---- SBUF pools --------------------------------------------------
        wt = es.enter_context(tc.tile_pool(name="wt", bufs=1))
        work = es.enter_context(tc.tile_pool(name="work", bufs=1))
        psC = es.enter_context(tc.tile_pool(name="psC", bufs=2, space="PSUM"))
        psS = es.enter_context(tc.tile_pool(name="psS", bufs=2, space="PSUM"))
        psB = es.enter_context(tc.tile_pool(name="psB", bufs=1, space="PSUM"))

        Iden = mybir.ActivationFunctionType.Identity
        Exp = mybir.ActivationFunctionType.Exp
        Sqrt = mybir.ActivationFunctionType.Sqrt
        ADD = mybir.AluOpType.add
        MUL = mybir.AluOpType.mult
        SUB = mybir.AluOpType.subtract
        BYP = mybir.AluOpType.bypass

        # ---- persistent inputs in SBUF ----------------------------------
        relT_sb, relT_free = tc.tile([P, 8 * 2 * K], bf16, name="relT_sb")
        hst_sb, hst_free = tc.tile([P, 8 * N], bf16, name="hst_sb")
        wp_sb = wt.tile([P, 5 * 8 * P], bf16, name="wp_sb", tag="wp_sb")
        nc.sync.dma_start(wp_sb[:, 0:16 * P], wpack[:, 0:16 * P])
        b8_sb = wt.tile([P, 8], f32, name="b8_sb", tag="b8_sb")
        nc.sync.dma_start(b8_sb[:], bias8[:])
        nc.sync.dma_start(hst_sb[:, 0:4 * N], hst[:, 0:4 * N])
        nc.sync.dma_start(hst_sb[:, 4 * N:8 * N], hst[:, 4 * N:8 * N])
        nc.sync.dma_start(wp_sb[:, 16 * P:40 * P], wpack[:, 16 * P:40 * P])
        nc.sync.dma_start(relT_sb[:, 0:8 * K], relT[:, 0:8 * K])
        nc.sync.dma_start(relT_sb[:, 8 * K:16 * K], relT[:, 8 * K:16 * K])
        gl_sb = wt.tile([1, 2 * HID], f32, name="gl_sb", tag="gl_sb")
        nc.sync.dma_start(gl_sb[:], gl[:])
        hsr_sb = wt.tile([P, HID], f32, name="hsr_sb", tag="hsr_sb")
        nc.sync.dma_start(hsr_sb[:], hsr[:])

        def wsl(kind, kt):
            base = (kind * 8 + kt) * P
            return wp_sb[:, base:base + P]

        def bcol(k):
            return b8_sb[:, k:k + 1]

        ident = wt.tile([P, P], skew_dt, name="ident", tag="ident")
        make_identity(nc, ident[:])

        ones1 = wt.tile([1, P], bf16, name="ones1", tag="ones1")
        nc.vector.memset(ones1[:], 1.0)
        gl_bf = wt.tile([1, 2 * HID], bf16, name="gl_bf", tag="gl_bf")
        nc.vector.tensor_copy(gl_bf[:], gl_sb[:])

        # ---- projections -------------------------------------------------
        # qT/kT: [128 (2 heads x 64 d), 1024 seq]
        qT = wt.tile([P, N], bf16, name="qT", tag="qT")
        kT = wt.tile([P, N], bf16, name="kT", tag="kT")

        def project_1024(dst, kind, rhs_sb, rhs_block, bias, rev=False):
            """dst[:, :1024] = sum_kt wsl(kind,kt).T @ rhs[kt block cols]."""
            ps = psS.tile([P, N], f32, name="pp", tag="st2")
            for c in range(2):
                for kt in range(8):
                    nc.tensor.matmul(ps[:, 512 * c:512 * (c + 1)], wsl(kind, kt),
                                     rhs_sb[:, rhs_block * 8192 + kt * N + 512 * c:
                                            rhs_block * 8192 + kt * N + 512 * (c + 1)],
                                     start=(kt == 0), stop=(kt == 7))
            if rev:
                ap = dst[:]
                rev_ap = bass.AP(ap.tensor, ap.offset + (N - 1),
                                 [[ap.ap[0][0], P], [-1, N]])
                nc.scalar.activation(rev_ap, ps[:], Iden, bias=bias)
            else:
                nc.scalar.activation(dst[:, 0:N], ps[:], Iden, bias=bias)

        project_1024(qT, 0, hst_sb, 0, bcol(0))
        project_1024(kT, 1, hst_sb, 0, bcol(1))

        # pos projections: pkT written column-REVERSED (== posk[2047-c]),
        # pqT normal. Both from the single normal relT.
        pkT = wt.tile([P, 2 * K], bf16, name="pkT", tag="pkT")
        pqT = wt.tile([P, 2 * K], bf16, name="pqT", tag="pqT")

        def project_pos(dst, kind, bias, rev):
            for half in range(2):
                ps = psS.tile([P, N], f32, name="pq", tag="st2")
                for c in range(2):
                    for kt in range(8):
                        col = kt * 2 * K + half * N + 512 * c
                        nc.tensor.matmul(ps[:, 512 * c:512 * (c + 1)],
                                         wsl(kind, kt),
                                         relT_sb[:, col:col + 512],
                                         start=(kt == 0), stop=(kt == 7))
                if rev:
                    # psum col c (global half*N + c) -> dst col 2047 - (half*N+c)
                    ap = dst[:]
                    base = ap.offset + (2 * K - 1 - half * N)
                    rev_ap = bass.AP(ap.tensor, base, [[ap.ap[0][0], P], [-1, N]])
                    nc.scalar.activation(rev_ap, ps[:], Iden, bias=bias)
                else:
                    nc.scalar.activation(dst[:, half * N:(half + 1) * N], ps[:],
                                         Iden, bias=bias)

        project_pos(pkT, 3, bcol(3), rev=True)
        project_pos(pqT, 4, bcol(4), rev=False)

        # va[jt]: [128 j, 132] = [v_h0(64) | one | pad | v_h1(64) | one | pad]
        # (projected after pk/pq so PE fills the c2p bounce window)
        va = []
        for jt in range(8):
            t = wt.tile([P, 132], bf16, name=f"va{jt}", tag=f"va{jt}")
            ps = psC.tile([P, 512], f32, name="pv", tag="pchunk")
            for kt in range(8):
                nc.tensor.matmul(ps[:, 0:DPC],
                                 hst_sb[:, kt * N + P * jt:kt * N + P * (jt + 1)],
                                 wsl(2, kt), start=(kt == 0), stop=(kt == 7))
            nc.scalar.copy(t[:, 0:64], ps[:, 0:64])
            nc.scalar.copy(t[:, 66:130], ps[:, 64:128])
            nc.vector.memset(t[:, 64:65], 1.0)
            nc.vector.memset(t[:, 130:131], 1.0)
            va.append(t)
        hst_free()
        relT_free()

        # ---- skew helper -------------------------------------------------
        evac_i = [0]

        def evac(dst_ap, src_ap):
            if evac_i[0] % 2 == 0:
                nc.vector.tensor_copy(dst_ap, src_ap)
            else:
                nc.scalar.copy(dst_ap, src_ap)
            evac_i[0] += 1

        def skew_group(lhsT, srcT, blocks, nm, gtag, gbufs):
            """blocks: list of (lhs_col0, hd_slice, w0). Returns gathered tile
            [128, len(blocks)*1024]: g[:, B*1024+x] = blk_B[p, 127-p+x]."""
            nb = len(blocks)
            blk = work.tile([P, nb * W_WIN], skew_dt, name=f"blk_{nm}",
                            tag=f"blk{nb}", bufs=2)
            for B, (c0l, hd, w0) in enumerate(blocks):
                for (c0, w) in ((0, 512), (512, 512), (1024, 127)):
                    ps = psC.tile([P, 512], f32, name="pblk", tag="pchunk")
                    nc.tensor.matmul(ps[:, 0:w],
                                     lhsT[hd, c0l:c0l + P],
                                     srcT[hd, w0 + c0:w0 + c0 + w],
                                     start=True, stop=True)
                    evac(blk[:, B * W_WIN + c0:B * W_WIN + c0 + w], ps[:, 0:w])
            scr = dwork.tile((P * nb * W_WIN,), skew_dt, name=f"scr_{nm}",
                             tag=f"scr{nb}", bufs=2)
            h = scr[:].tensor
            nc.sync.dma_start(
                bass.AP(h, scr[:].offset, [[nb * W_WIN, P], [1, nb * W_WIN]]),
                blk[:])
            g = work.tile([P, nb * N], skew_dt, name=f"g_{nm}", tag=gtag,
                          bufs=gbufs)
            nc.sync.dma_start(
                g[:], bass.AP(h, scr[:].offset + 127,
                              [[nb * W_WIN - 1, P], [W_WIN, nb], [1, N]]))
            return g

        # ---- c2p gather: 4 groups of (2 i-tiles x 2 heads) ---------------
        # block (r, h): lhsT=qT[hd, 128r:...], src=pkT(rev), w0 = 896-128r
        g_c = []
        for grp in range(4):
            blocks = []
            for dr in range(2):
                r = 2 * grp + dr
                for h in range(HPC):
                    hd = slice(64 * h, 64 * h + 64)
                    blocks.append((P * r, hd, 896 - 128 * r))
            g_c.append(skew_group(qT, pkT, blocks, f"c{grp}", f"g_c{grp}", 1))

        # ---- per-head attention -----------------------------------------
        ctxT = wt.tile([P, N], bf16, name="ctxT", tag="ctxT")

        def c2p_slice(r, h, jt):
            g = g_c[r // 2]
            B = 2 * (r % 2) + h
            return g[:, B * N + P * jt:B * N + P * (jt + 1)]

        for h in range(HPC):
            hd = slice(64 * h, 64 * h + 64)
            pb = psB.tile([65, N], f32, name="pb", tag="pb")
            g_p = {}
            for jt in range(8):
                if jt % 2 == 0:
                    blocks = [(P * (jt + dj), hd, 897 - 128 * (jt + dj))
                              for dj in range(2)]
                    g_p[jt // 2] = skew_group(kT, pqT, blocks,
                                              f"p{h}_{jt // 2}", "g_p", 2)
                p2cT = g_p[jt // 2]
                pB = jt % 2
                st = psS.tile([P, N], f32, name="st", tag="st2")
                for c in range(2):
                    nc.tensor.matmul(st[:, 512 * c:512 * (c + 1)],
                                     kT[hd, P * jt:P * (jt + 1)],
                                     qT[hd, 512 * c:512 * (c + 1)],
                                     start=True, stop=False)
                    for rr in range(4):
                        r = 4 * c + rr
                        nc.tensor.matmul(st[:, 512 * c + P * rr:512 * c + P * (rr + 1)],
                                         c2p_slice(r, h, jt), ident[:],
                                         start=False, stop=(rr == 3))
                s2 = work.tile([P, N], f32, name="s2", tag="s2", bufs=2)
                nc.vector.tensor_add(s2[:], st[:], p2cT[:, pB * N:(pB + 1) * N])
                e = work.tile([P, N], bf16, name="e", tag="e", bufs=2)
                nc.scalar.activation(e[:], s2[:], Exp, scale=SCALE)
                for c in range(2):
                    nc.tensor.matmul(pb[:, 512 * c:512 * (c + 1)],
                                     va[jt][:, 66 * h:66 * h + 65],
                                     e[:, 512 * c:512 * (c + 1)],
                                     start=(jt == 0), stop=(jt == 7))

            # normalize: ctxT[hd] = pb[0:64] * (1/pb[64]) + bv
            rec = work.tile([1, N], bf16, name="rec", tag="rec", bufs=2)
            with nc.allow_low_precision(reason="bf16 softmax-recip broadcast"):
                nc.vector.reciprocal(rec[:], pb[64:65, :])
            rbc = psS.tile([P, N], f32, name="rbc", tag="st2")
            for c in range(2):
                nc.tensor.matmul(rbc[0:64, 512 * c:512 * (c + 1)], ones1[:, 0:64],
                                 rec[:, 512 * c:512 * (c + 1)],
                                 start=True, stop=True)
            rbs = work.tile([64, N], bf16, name="rbs", tag="rbs", bufs=2)
            nc.scalar.copy(rbs[:], rbc[0:64, :])
            ctmp = work.tile([64, N], bf16, name="ctmp", tag="ctmp", bufs=2)
            nc.vector.tensor_mul(ctmp[:], pb[0:64, :], rbs[:])
            nc.scalar.activation(ctxT[hd, :], ctmp[:], Iden, bias=b8_sb[hd, 2:3])


        wot_sb, _wot_free = tc.tile([P, HID], bf16, name="wot_sb")
        nc.sync.dma_start(wot_sb[:], wot[:])

        # ---- output dense partials over all rows -> ReduceScatter --------
        opart = dwork.tile((N, HID), bf16, name="opart", tag="opart")
        rs_out = dwork.tile((P, HID), bf16, name="rs_out", tag="rs_out")
        for it in range(8):
            osb = work.tile([P, HID], bf16, name="osb", tag="osb", bufs=2)
            for c in range(2):
                po = psC.tile([P, 512], f32, name="po", tag="pchunk")
                nc.tensor.matmul(po[:], ctxT[:, P * it:P * (it + 1)],
                                 wot_sb[:, 512 * c:512 * (c + 1)],
                                 start=True, stop=True)
                nc.scalar.copy(osb[:, 512 * c:512 * (c + 1)], po[:])
            nc.sync.dma_start(opart[P * it:P * (it + 1), :], osb[:])
        nc.gpsimd.collective_compute(
            "ReduceScatter", ADD, replica_groups=[list(range(NCORES))],
            ins=[opart[:]], outs=[rs_out[:]])

        # ---- ln_g / ln_b broadcast via K=1 matmul ------------------------
        g_sb, _gf = tc.tile([P, HID], f32, name="g_sb")
        b_sb, _bf = tc.tile([P, HID], f32, name="b_sb")
        for half, dst in ((0, g_sb), (1, b_sb)):
            pg = psS.tile([P, N], f32, name="pg", tag="st2")
            for c in range(2):
                nc.tensor.matmul(pg[:, 512 * c:512 * (c + 1)], ones1[:],
                                 gl_bf[:, half * HID + 512 * c:half * HID + 512 * (c + 1)],
                                 start=True, stop=True)
            nc.vector.tensor_copy(dst[:], pg[:])

        # ---- residual + LayerNorm on own 128 rows ------------------------
        xr, _xrf = tc.tile([P, HID], f32, name="xr")
        nc.gpsimd.dma_start(xr[:], rs_out[:])
        x, _xf = tc.tile([P, HID], f32, name="x")
        nc.vector.tensor_add(x[:], xr[:], hsr_sb[:])

        stats = wt.tile([P, 2, 6], f32, name="stats", tag="stats")
        mv = wt.tile([P, 2], f32, name="mv", tag="mv")
        for s in range(2):
            nc.vector.bn_stats(stats[:, s, :], x[:, 512 * s:512 * (s + 1)])
        nc.vector.bn_aggr(mv[:], stats[:])
        epsb = wt.tile([P, 1], f32, name="epsb", tag="epsb")
        nc.vector.memset(epsb[:], EPS)
        std = wt.tile([P, 1], f32, name="std", tag="std")
        nc.scalar.activation(std[:], mv[:, 1:2], Sqrt, bias=epsb[:])
        rstd = wt.tile([P, 1], f32, name="rstd", tag="rstd")
        nc.vector.reciprocal(rstd[:], std[:])

        t1, _t1f = tc.tile([P, HID], f32, name="t1")
        nc.vector.scalar_tensor_tensor(t1[:], x[:], mv[:, 0:1], g_sb[:],
                                       op0=SUB, op1=MUL)
        yout, _yf = tc.tile([P, HID], f32, name="yout")
        nc.vector.scalar_tensor_tensor(yout[:], t1[:], rstd[:], b_sb[:],
                                       op0=MUL, op1=ADD)
        nc.sync.dma_start(out_t[:], yout[:])
        for f in (_yf, _t1f, _xf, _xrf, _bf, _gf, _wot_free):
            f()

    nc.compile()
    return nc, names


def _get_compiled():
    if "nc" not in _CACHE:
        nc, names = _build()
        _CACHE["nc"] = nc
        _CACHE["names"] = names
    return _CACHE["nc"], _CACHE["names"]


def _pack8(mat, width):
    # (1024, width) -> [128, 8*width]: out[p, kt*width+c] = mat[128*kt+p, c]
    return np.ascontiguousarray(
        mat.reshape(8, P, width).transpose(1, 0, 2).reshape(P, 8 * width))


def _prep_in_maps(inputs):
    import ml_dtypes

    bf = ml_dtypes.bfloat16
    hs = np.asarray(inputs["hidden_states"], np.float32)[0]      # (N, HID)
    rel = np.asarray(inputs["rel_embeddings"], np.float32)       # (2K, HID)
    hst = _pack8(np.ascontiguousarray(hs.T), N).astype(bf)
    relT = _pack8(np.ascontiguousarray(rel.T), 2 * K).astype(bf)
    Wo = np.asarray(inputs["Wo"], np.float32)

    kinds = ["Wq", "Wk", "Wv", "Wpk", "Wpq"]
    bias_names = ["bq", "bk", "bv", "bpk", "bpq"]

    in_maps = []
    for r in range(NCORES):
        wpack = np.zeros((P, 5 * 8 * P), np.float32)
        for ki, kn in enumerate(kinds):
            w = np.asarray(inputs[kn], np.float32)
            wt = np.ascontiguousarray(w[DPC * r:DPC * (r + 1), :].T)  # (1024,128)
            wpack[:, ki * 8 * P:(ki + 1) * 8 * P] = _pack8(wt, P)
        bias8 = np.zeros((P, 8), np.float32)
        for bi, bn in enumerate(bias_names):
            bias8[:, bi] = np.asarray(inputs[bn], np.float32)[DPC * r:DPC * (r + 1)]
        glrow = np.concatenate([np.asarray(inputs["ln_g"], np.float32),
                                np.asarray(inputs["ln_b"], np.float32)])[None, :]
        hsr = np.ascontiguousarray(hs[P * r:P * (r + 1), :]) + \
            np.asarray(inputs["bo"], np.float32)[None, :]
        m = {
            "hst": hst,
            "relT": relT,
            "wpack": wpack.astype(bf),
            "wot": np.ascontiguousarray(
                Wo[:, DPC * r:DPC * (r + 1)].T).astype(bf),
            "hsr": np.ascontiguousarray(hsr.astype(np.float32)),
            "bias8": bias8,
            "gl": np.ascontiguousarray(glrow),
        }
        in_maps.append(m)
    return in_maps


def run(inputs, trace=False):
    from concourse.bass_utils import run_bass_kernel_spmd

    nc, names = _get_compiled()
    logical = _prep_in_maps(inputs)
    in_maps = [{names[k]: v for k, v in m.items()} for m in logical]
    res = run_bass_kernel_spmd(nc, in_maps, list(range(NCORES)), trace=trace)
    outs = [res.results[r][names["out"]].astype(np.float32) for r in range(NCORES)]
    full = np.concatenate(outs, axis=0).reshape(1, N, HID)
    return full, res


def kernel(**inputs) -> np.ndarray:
    full, _ = run(inputs, trace=False)
    return full
